# revision 22
# baseline (speedup 1.0000x reference)
# Trainium2 Bass kernel for masked causal attention
#   B=2, H=16, S=2048, D=64, bool attn_mask [B, S, S] + causal, softmax, @V.
#
# Sharding: 8 cores x 4 heads (cores 0-3 -> batch 0, cores 4-7 -> batch 1).
#
# Softmax numerator/denominator are computed unnormalized on device (ones-row
# in the PV lhsT gives the denominator row); the final divide happens on HOST.
#
# Each head is processed in TWO q-passes of 1024 columns. That keeps the PV
# accumulator at 2 PSUM banks (instead of 4), freeing one bank as a target
# for dependency-free "HAM keep-alive" dummy matmuls: the PE's clock gate
# (HAM) re-throttles 2.4GHz -> 1.2GHz whenever the PE micro-idles, and the
# score-ring WAR stalls (QK chunk N waits for the drain of chunk N-2) were
# costing ~50us/core of cold-clock matmuls. A dummy matmul emitted after
# every QK unit keeps the array active across those stalls.
#
# Per (head, pass, k-tile j), chunk = [max(128j, 1024p), 1024(p+1)):
#  - k-tiles 0..3 ("path B"): ACT exp reads the f32 score chunk straight
#    from PSUM; the 0/1 mask is applied multiplicatively on fp16 (GPSIMD for
#    j<2, DVE for j 2..3).
#  - k-tiles 4..15 ("path A"): DVE drains PSUM with a fused additive mask
#    bias (0 keep / -100 masked) into causal-packed fp16 group buffers; ACT
#    exps each group in ONE wide in-place instruction.
# PV accumulates [V | ones] @ p per 512-wide q-bank into f32 PSUM; banks
# drain via DVE as they complete. PV matmuls of a section are interleaved
# into the next section's QK stream to keep the PE dense.

import os
import numpy as np

B, H, S, D = 2, 16, 2048, 64
NCORES = 8
HPC = 4          # heads per core
P = 128
NKT = S // P     # 16 k-tiles
DP1 = D + 1      # 64 value rows + denominator ones-row
GS = 4           # k-tiles per wide-exp group (path A)
BJ = 4           # k-tiles on path B
PASS_W = 1024
GPSIMD_BJ = 4    # path-B k-tiles j < this get their mask mult on GPSIMD

PADQ = int(os.environ.get("ATTN_PADQ", "512"))   # dummy-MM cols per QK unit
PADN = int(os.environ.get("ATTN_PADN", "2"))     # dummy MMs per QK unit


def pass_tiles(p):
    return [j for j in range(NKT) if P * j < PASS_W * (p + 1)]


def pchunk(j, p):
    return (max(P * j, PASS_W * p), PASS_W * (p + 1))


# causal-packed mask layouts, per (pass, k-tile)
M01OFF, MBOFF, SPW, SPLOC = {}, {}, {}, {}
_off = 0
for _p in (0, 1):
    for _j in range(BJ):
        c0, e = pchunk(_j, _p)
        M01OFF[(_p, _j)] = _off
        _off += e - c0
M01W = _off                                       # 7424
_off = 0
for _p in (0, 1):
    for _j in pass_tiles(_p):
        if _j < BJ:
            continue
        c0, e = pchunk(_j, _p)
        g = _j // GS
        if (_p, g) not in SPW:
            SPW[(_p, g)] = 0
        MBOFF[(_p, _j)] = _off
        SPLOC[(_p, _j)] = SPW[(_p, g)]
        SPW[(_p, g)] += e - c0
        _off += e - c0
MBW = _off                                        # 9984

_cache = {}


def build_nc():
    import concourse.bacc as bacc
    import concourse.mybir as mybir
    import concourse.tile as tile
    from concourse import library_config
    from contextlib import ExitStack

    fp16 = mybir.dt.float16
    f32 = mybir.dt.float32
    Exp = mybir.ActivationFunctionType.Exp

    nc = bacc.Bacc("TRN2", target_bir_lowering=False, debug=False,
                   num_devices=NCORES)

    qt_d = nc.dram_tensor("qt", [HPC, D, S], fp16, kind="ExternalInput")
    kt_d = nc.dram_tensor("kt", [HPC, D, S], fp16, kind="ExternalInput")
    vp_d = nc.dram_tensor("vp", [HPC, P, NKT, DP1], fp16, kind="ExternalInput")
    m01_d = nc.dram_tensor("m01", [P, M01W], fp16, kind="ExternalInput")
    mb_d = nc.dram_tensor("mb", [P, MBW], fp16, kind="ExternalInput")
    out_d = nc.dram_tensor("outt", [HPC, DP1, S], f32, kind="ExternalOutput")

    with tile.TileContext(nc) as tc, ExitStack() as ctx:
        mk_pool = ctx.enter_context(tc.tile_pool(name="mk", bufs=1))
        qk_pool = ctx.enter_context(tc.tile_pool(name="qk", bufs=2))
        vp_pool = ctx.enter_context(tc.tile_pool(name="vpool", bufs=2))
        sp_pool = ctx.enter_context(tc.tile_pool(name="sp", bufs=2))
        pb_pool = ctx.enter_context(tc.tile_pool(name="pb", bufs=8))
        osb_pool = ctx.enter_context(tc.tile_pool(name="osb", bufs=4))
        warm_pool = ctx.enter_context(tc.tile_pool(name="warm", bufs=1))
        st_psum = ctx.enter_context(tc.tile_pool(name="st", bufs=2, space="PSUM"))
        o_psum = ctx.enter_context(tc.tile_pool(name="outp", bufs=1, space="PSUM"))

        nc.gpsimd.load_library(library_config.standard)

        # dummy bank: PE-only, nothing reads it -> dummy MMs have no deps
        wsb = warm_pool.tile([P, 512], fp16, tag="warm")
        nc.vector.memset(wsb[:], 0.0)
        dummy_ps = o_psum.tile([P, 512], f32, tag="dummy")
        # PE warm-up: dense matmuls on zeros open the HAM gate at kernel start
        for _ in range(12):
            nc.tensor.matmul(dummy_ps[:], lhsT=wsb[:, 0:128], rhs=wsb[:],
                             start=True, stop=True)

        def pad(n=None):
            for _ in range(PADN if n is None else n):
                if PADQ:
                    nc.tensor.matmul(dummy_ps[:, 0:PADQ],
                                     lhsT=wsb[0:64, 0:128],
                                     rhs=wsb[0:64, 0:PADQ],
                                     start=True, stop=True)

        def load_head(h):
            qt = qk_pool.tile([D, S], fp16, tag="qt")
            nc.sync.dma_start(qt[:], qt_d[h])
            kt = qk_pool.tile([D, S], fp16, tag="kt")
            nc.sync.dma_start(kt[:], kt_d[h])
            vp = vp_pool.tile([P, NKT, DP1], fp16, tag="vp")
            nc.sync.dma_start(vp[:], vp_d[h])
            return qt, kt, vp

        head_tiles = {0: load_head(0)}
        m01_sb = mk_pool.tile([P, M01W], fp16, tag="m01")
        mb_sb = mk_pool.tile([P, MBW], fp16, tag="mb")
        for (p_, j_), o_ in sorted(M01OFF.items(), key=lambda kv: kv[1]):
            c0, e = pchunk(j_, p_)
            nc.sync.dma_start(m01_sb[:, o_:o_ + e - c0], m01_d[:, o_:o_ + e - c0])
        for (p_, j_), o_ in sorted(MBOFF.items(), key=lambda kv: kv[1]):
            c0, e = pchunk(j_, p_)
            nc.sync.dma_start(mb_sb[:, o_:o_ + e - c0], mb_d[:, o_:o_ + e - c0])

        def qk_units(h, p, qt, kt, vp, sp_tiles, pb_tiles):
            """One callable per k-tile chunk of this pass."""
            def unit(j):
                def run():
                    c0, e = pchunk(j, p)
                    w = e - c0
                    st = st_psum.tile([P, PASS_W], f32, tag="st",
                                      name=f"st_h{h}p{p}j{j}")
                    for lo in range(0, w, 512):
                        wl = min(512, w - lo)
                        nc.tensor.matmul(
                            st[:, lo:lo + wl],
                            lhsT=kt[:, j * P:(j + 1) * P],
                            rhs=qt[:, c0 + lo:c0 + lo + wl],
                            start=True, stop=True)
                    # small A-chunks have short drain stalls -> 1 pad suffices
                    pad(PADN if (j < BJ or w >= 768) else min(PADN, 1))
                    if j < BJ:
                        # path B: chunk exp from PSUM, then 0/1 mask mult
                        pb = pb_pool.tile([P, PASS_W], fp16, tag="pb",
                                          name=f"pb_h{h}p{p}j{j}")
                        nc.scalar.activation(pb[:, :w], st[:, :w], Exp)
                        mo = M01OFF[(p, j)]
                        eng = nc.gpsimd if j < GPSIMD_BJ else nc.vector
                        eng.tensor_mul(pb[:, :w], pb[:, :w],
                                       m01_sb[:, mo:mo + w])
                        pb_tiles[(p, j)] = pb
                    else:
                        # path A: fused drain + additive mask bias
                        g = j // GS
                        if (p, g) not in sp_tiles:
                            sp_tiles[(p, g)] = sp_pool.tile(
                                [P, SPW[(p, g)]], fp16, tag=f"sp{p}{g}",
                                name=f"sp_h{h}p{p}g{g}")
                        lo = SPLOC[(p, j)]
                        mo = MBOFF[(p, j)]
                        nc.vector.tensor_add(sp_tiles[(p, g)][:, lo:lo + w],
                                             st[:, :w],
                                             mb_sb[:, mo:mo + w])
                        if j == max(jj for jj in pass_tiles(p)
                                    if jj // GS == g):
                            # wide in-place exps for the whole group, split
                            # <=2048 so ACT FIFO head-blocking of the score
                            # drains stays bounded (~2us, not ~4us)
                            spw = SPW[(p, g)]
                            for elo in range(0, spw, 2048):
                                ew = min(2048, spw - elo)
                                nc.scalar.activation(
                                    sp_tiles[(p, g)][:, elo:elo + ew],
                                    sp_tiles[(p, g)][:, elo:elo + ew], Exp)
                return run
            return [unit(j) for j in pass_tiles(p)]

        def pv_units(h, p, vp, sp_tiles, pb_tiles):
            """PV MMs (j ascending => per-bank start..stop order) plus the
            per-bank drain right after the bank's last MM."""
            outp = {}
            units = []

            def mk_mm(j, lb):
                def run():
                    qb = 2 * p + lb
                    if lb not in outp:
                        outp[lb] = o_psum.tile([DP1, 512], f32,
                                               tag=f"outp{lb}",
                                               name=f"outp_h{h}p{p}b{lb}")
                    q0 = max(P * j, 512 * qb)
                    q1 = 512 * (qb + 1)
                    c0 = pchunk(j, p)[0]
                    if j < BJ:
                        rhs = pb_tiles[(p, j)][:, q0 - c0:q1 - c0]
                    else:
                        g = j // GS
                        lo = SPLOC[(p, j)] + (q0 - c0)
                        rhs = sp_tiles[(p, g)][:, lo:lo + (q1 - q0)]
                    nc.tensor.matmul(
                        outp[lb][:, q0 - 512 * qb:q1 - 512 * qb],
                        lhsT=vp[:, j, :], rhs=rhs,
                        start=(j == 0),
                        stop=(j == min(4 * qb + 3, NKT - 1)))
                return run

            def mk_drain(lb):
                def run():
                    qb = 2 * p + lb
                    osb = osb_pool.tile([DP1, 512], f32, tag="osb",
                                        name=f"osb_h{h}p{p}b{lb}")
                    nc.vector.tensor_copy(osb[:], outp[lb][:])
                    nc.sync.dma_start(out_d[h, :, 512 * qb:512 * (qb + 1)],
                                      osb[:])
                return run

            for j in pass_tiles(p):
                for lb in (0, 1):
                    qb = 2 * p + lb
                    if j > min(4 * qb + 3, NKT - 1):
                        continue
                    units.append(mk_mm(j, lb))
                    if j == min(4 * qb + 3, NKT - 1):
                        units.append(mk_drain(lb))
            return units

        def interleave(qk, pv):
            """Emit QK units with prior-section pv callables spread between."""
            done = 0
            for i, u in enumerate(qk):
                u()
                want = (i + 1) * len(pv) // len(qk)
                while done < want:
                    pv[done]()
                    done += 1
            while done < len(pv):
                pv[done]()
                done += 1

        prev_pv = []
        for h in range(HPC):
            qt, kt, vp = head_tiles.pop(h, None) or load_head(h)
            sp_tiles, pb_tiles = {}, {}
            for p in (0, 1):
                interleave(qk_units(h, p, qt, kt, vp, sp_tiles, pb_tiles),
                           prev_pv)
                prev_pv = pv_units(h, p, vp, sp_tiles, pb_tiles)
            if h + 1 < HPC:
                head_tiles[h + 1] = load_head(h + 1)
        for u in prev_pv:
            u()

    nc.compile()
    return nc


def prep_inputs(query, key, value, attn_mask):
    """Host-side layout prep (transposes/retiling/casts only) -> 8 in_maps."""
    query = np.asarray(query, dtype=np.float32)
    key = np.asarray(key, dtype=np.float32)
    value = np.asarray(value, dtype=np.float32)
    attn_mask = np.asarray(attn_mask).astype(bool)

    # fold the 1/sqrt(D)=0.125 softmax scale into Q
    qT = np.ascontiguousarray(
        (query * 0.125).transpose(0, 1, 3, 2)).astype(np.float16)
    kT = np.ascontiguousarray(key.transpose(0, 1, 3, 2)).astype(np.float16)

    vp = np.concatenate(
        [value, np.ones((B, H, S, 1), np.float32)], axis=3).astype(np.float16)
    # [B, H, S, 65] -> [B, H, 128, NKT, 65] (partition-contiguous tiles)
    vp = np.ascontiguousarray(
        vp.reshape(B, H, NKT, P, DP1).transpose(0, 1, 3, 2, 4))

    tril = np.tril(np.ones((S, S), dtype=bool))
    in_maps = []
    for b in range(B):
        m = (attn_mask[b] & tril)          # [q, k] True = keep
        mT = m.T                           # [k, q]
        m01 = np.empty((P, M01W), np.float16)
        for (p, j), o in M01OFF.items():
            c0, e = pchunk(j, p)
            m01[:, o:o + e - c0] = mT[P * j:P * (j + 1), c0:e]
        mb = np.empty((P, MBW), np.float16)
        for (p, j), o in MBOFF.items():
            c0, e = pchunk(j, p)
            mb[:, o:o + e - c0] = np.where(mT[P * j:P * (j + 1), c0:e],
                                           np.float16(0.0), np.float16(-100.0))
        for cl in range(NCORES // B):
            h0 = cl * HPC
            in_maps.append({
                "qt": np.ascontiguousarray(qT[b, h0:h0 + HPC]),
                "kt": np.ascontiguousarray(kT[b, h0:h0 + HPC]),
                "vp": np.ascontiguousarray(vp[b, h0:h0 + HPC]),
                "m01": m01,
                "mb": mb,
            })
    return in_maps


def run(query, key, value, attn_mask, trace=False, trace_cores=None):
    from concourse import bass_utils

    if "nc" not in _cache:
        _cache["nc"] = build_nc()
    nc = _cache["nc"]

    in_maps = prep_inputs(query, key, value, attn_mask)
    res = bass_utils.run_bass_kernel_spmd(
        nc, in_maps, core_ids=list(range(NCORES)),
        trace=trace, trace_cores=trace_cores)

    out = np.empty((B, H, S, D), np.float32)
    for c in range(NCORES):
        b = c // (NCORES // B)
        h0 = (c % (NCORES // B)) * HPC
        outt = res.results[c]["outt"]          # [HPC, 65, S]
        num = outt[:, 0:D, :]                  # [HPC, 64, S]
        den = outt[:, D:D + 1, :]              # [HPC, 1, S]
        out[b, h0:h0 + HPC] = (num / den).transpose(0, 2, 1)
    return out, res


def kernel(query, key, value, attn_mask):
    out, _ = run(query, key, value, attn_mask)
    return out


# revision 23
# speedup vs baseline: 1.1219x; 1.1219x over previous
# Trainium2 Bass kernel for masked causal attention
#   B=2, H=16, S=2048, D=64, bool attn_mask [B, S, S] + causal, softmax, @V.
#
# Sharding: 8 cores x 4 heads (cores 0-3 -> batch 0, cores 4-7 -> batch 1).
#
# Softmax numerator/denominator are computed unnormalized on device (ones-row
# in the PV lhsT gives the denominator row); the final divide happens on HOST.
#
# Each head is processed in TWO q-passes of 1024 columns. That keeps the PV
# accumulator at 2 PSUM banks (instead of 4), freeing one bank as a target
# for dependency-free "HAM keep-alive" dummy matmuls: the PE's clock gate
# (HAM) re-throttles 2.4GHz -> 1.2GHz whenever the PE micro-idles, and the
# score-ring WAR stalls (QK chunk N waits for the drain of chunk N-2) were
# costing ~50us/core of cold-clock matmuls. A dummy matmul emitted after
# every QK unit keeps the array active across those stalls.
#
# Per (head, pass, k-tile j), chunk = [max(128j, 1024p), 1024(p+1)):
#  - k-tiles 0..3 ("path B"): ACT exp reads the f32 score chunk straight
#    from PSUM; the 0/1 mask is applied multiplicatively on fp16 (GPSIMD for
#    j<2, DVE for j 2..3).
#  - k-tiles 4..15 ("path A"): DVE drains PSUM with a fused additive mask
#    bias (0 keep / -100 masked) into causal-packed fp16 group buffers; ACT
#    exps each group in ONE wide in-place instruction.
# PV accumulates [V | ones] @ p per 512-wide q-bank into f32 PSUM; banks
# drain via DVE as they complete. PV matmuls of a section are interleaved
# into the next section's QK stream to keep the PE dense.

import os
import numpy as np

B, H, S, D = 2, 16, 2048, 64
NCORES = 8
HPC = 4          # heads per core
P = 128
NKT = S // P     # 16 k-tiles
DP1 = D + 1      # 64 value rows + denominator ones-row
GS = 4           # k-tiles per wide-exp group (path A)
BJ = 4           # k-tiles on path B
PASS_W = 1024
GPSIMD_BJ = 4    # path-B k-tiles j < this get their mask mult on GPSIMD

PADQ = int(os.environ.get("ATTN_PADQ", "512"))   # dummy-MM cols per QK unit
PADN = int(os.environ.get("ATTN_PADN", "2"))     # dummy MMs per QK unit


def pass_tiles(p):
    return [j for j in range(NKT) if P * j < PASS_W * (p + 1)]


def pchunk(j, p):
    return (max(P * j, PASS_W * p), PASS_W * (p + 1))


# causal-packed mask layouts, per (pass, k-tile)
M01OFF, MBOFF, SPW, SPLOC = {}, {}, {}, {}
_off = 0
for _p in (0, 1):
    for _j in range(BJ):
        c0, e = pchunk(_j, _p)
        M01OFF[(_p, _j)] = _off
        _off += e - c0
M01W = _off                                       # 7424
_off = 0
for _p in (0, 1):
    for _j in pass_tiles(_p):
        if _j < BJ:
            continue
        c0, e = pchunk(_j, _p)
        g = _j // GS
        if (_p, g) not in SPW:
            SPW[(_p, g)] = 0
        MBOFF[(_p, _j)] = _off
        SPLOC[(_p, _j)] = SPW[(_p, g)]
        SPW[(_p, g)] += e - c0
        _off += e - c0
MBW = _off                                        # 9984

_cache = {}


def build_nc():
    import concourse.bacc as bacc
    import concourse.mybir as mybir
    import concourse.tile as tile
    from concourse import library_config
    from contextlib import ExitStack

    fp16 = mybir.dt.float16
    f32 = mybir.dt.float32
    Exp = mybir.ActivationFunctionType.Exp

    nc = bacc.Bacc("TRN2", target_bir_lowering=False, debug=False,
                   num_devices=NCORES)

    qt_d = nc.dram_tensor("qt", [HPC, D, S], fp16, kind="ExternalInput")
    kt_d = nc.dram_tensor("kt", [HPC, D, S], fp16, kind="ExternalInput")
    vp_d = nc.dram_tensor("vp", [HPC, P, NKT, DP1], fp16, kind="ExternalInput")
    m01_d = nc.dram_tensor("m01", [P, M01W], fp16, kind="ExternalInput")
    mb_d = nc.dram_tensor("mb", [P, MBW], fp16, kind="ExternalInput")
    out_d = nc.dram_tensor("outt", [HPC, DP1, S], f32, kind="ExternalOutput")

    with tile.TileContext(nc) as tc, ExitStack() as ctx:
        mk_pool = ctx.enter_context(tc.tile_pool(name="mk", bufs=1))
        qk_pool = ctx.enter_context(tc.tile_pool(name="qk", bufs=2))
        vp_pool = ctx.enter_context(tc.tile_pool(name="vpool", bufs=2))
        sp_pool = ctx.enter_context(tc.tile_pool(name="sp", bufs=2))
        pb_pool = ctx.enter_context(tc.tile_pool(name="pb", bufs=8))
        osb_pool = ctx.enter_context(tc.tile_pool(name="osb", bufs=4))
        warm_pool = ctx.enter_context(tc.tile_pool(name="warm", bufs=1))
        st_psum = ctx.enter_context(tc.tile_pool(name="st", bufs=2, space="PSUM"))
        o_psum = ctx.enter_context(tc.tile_pool(name="outp", bufs=1, space="PSUM"))

        nc.gpsimd.load_library(library_config.standard)

        # dummy bank: PE-only, nothing reads it -> dummy MMs have no deps
        wsb = warm_pool.tile([P, 512], fp16, tag="warm")
        nc.vector.memset(wsb[:], 0.0)
        dummy_ps = o_psum.tile([P, 512], f32, tag="dummy")
        # PE warm-up: dense matmuls on zeros open the HAM gate at kernel start
        for _ in range(12):
            nc.tensor.matmul(dummy_ps[:], lhsT=wsb[:, 0:128], rhs=wsb[:],
                             start=True, stop=True)

        def pad(n=None):
            for _ in range(PADN if n is None else n):
                if PADQ:
                    nc.tensor.matmul(dummy_ps[:, 0:PADQ],
                                     lhsT=wsb[0:64, 0:128],
                                     rhs=wsb[0:64, 0:PADQ],
                                     start=True, stop=True)

        def load_head(h):
            qt = qk_pool.tile([D, S], fp16, tag="qt")
            nc.sync.dma_start(qt[:], qt_d[h])
            kt = qk_pool.tile([D, S], fp16, tag="kt")
            nc.sync.dma_start(kt[:], kt_d[h])
            vp = vp_pool.tile([P, NKT, DP1], fp16, tag="vp")
            nc.sync.dma_start(vp[:], vp_d[h])
            return qt, kt, vp

        head_tiles = {0: load_head(0)}
        m01_sb = mk_pool.tile([P, M01W], fp16, tag="m01")
        mb_sb = mk_pool.tile([P, MBW], fp16, tag="mb")
        for (p_, j_), o_ in sorted(M01OFF.items(), key=lambda kv: kv[1]):
            c0, e = pchunk(j_, p_)
            nc.sync.dma_start(m01_sb[:, o_:o_ + e - c0], m01_d[:, o_:o_ + e - c0])
        for (p_, j_), o_ in sorted(MBOFF.items(), key=lambda kv: kv[1]):
            c0, e = pchunk(j_, p_)
            nc.sync.dma_start(mb_sb[:, o_:o_ + e - c0], mb_d[:, o_:o_ + e - c0])

        def qk_units(h, p, qt, kt, vp, sp_tiles, pb_tiles):
            """One callable per k-tile chunk of this pass."""
            def unit(j):
                def run():
                    c0, e = pchunk(j, p)
                    w = e - c0
                    st = st_psum.tile([P, PASS_W], f32, tag="st",
                                      name=f"st_h{h}p{p}j{j}")
                    for lo in range(0, w, 512):
                        wl = min(512, w - lo)
                        nc.tensor.matmul(
                            st[:, lo:lo + wl],
                            lhsT=kt[:, j * P:(j + 1) * P],
                            rhs=qt[:, c0 + lo:c0 + lo + wl],
                            start=True, stop=True)
                    pad()
                    if j < BJ:
                        # path B: chunk exp from PSUM, then 0/1 mask mult
                        pb = pb_pool.tile([P, PASS_W], fp16, tag="pb",
                                          name=f"pb_h{h}p{p}j{j}")
                        nc.scalar.activation(pb[:, :w], st[:, :w], Exp)
                        mo = M01OFF[(p, j)]
                        eng = nc.gpsimd if j < GPSIMD_BJ else nc.vector
                        eng.tensor_mul(pb[:, :w], pb[:, :w],
                                       m01_sb[:, mo:mo + w])
                        pb_tiles[(p, j)] = pb
                    else:
                        # path A: fused drain + additive mask bias
                        g = j // GS
                        if (p, g) not in sp_tiles:
                            sp_tiles[(p, g)] = sp_pool.tile(
                                [P, SPW[(p, g)]], fp16, tag=f"sp{p}{g}",
                                name=f"sp_h{h}p{p}g{g}")
                        lo = SPLOC[(p, j)]
                        mo = MBOFF[(p, j)]
                        nc.vector.tensor_add(sp_tiles[(p, g)][:, lo:lo + w],
                                             st[:, :w],
                                             mb_sb[:, mo:mo + w])
                        if j == max(jj for jj in pass_tiles(p)
                                    if jj // GS == g):
                            # wide in-place exps for the whole group, split
                            # <=2048 so ACT FIFO head-blocking of the score
                            # drains stays bounded (~2us, not ~4us)
                            spw = SPW[(p, g)]
                            for elo in range(0, spw, 2048):
                                ew = min(2048, spw - elo)
                                nc.scalar.activation(
                                    sp_tiles[(p, g)][:, elo:elo + ew],
                                    sp_tiles[(p, g)][:, elo:elo + ew], Exp)
                return run
            return [unit(j) for j in pass_tiles(p)]

        def pv_units(h, p, vp, sp_tiles, pb_tiles):
            """PV MMs (j ascending => per-bank start..stop order) plus the
            per-bank drain right after the bank's last MM."""
            outp = {}
            units = []

            def mk_mm(j, lb):
                def run():
                    qb = 2 * p + lb
                    if lb not in outp:
                        outp[lb] = o_psum.tile([DP1, 512], f32,
                                               tag=f"outp{lb}",
                                               name=f"outp_h{h}p{p}b{lb}")
                    q0 = max(P * j, 512 * qb)
                    q1 = 512 * (qb + 1)
                    c0 = pchunk(j, p)[0]
                    if j < BJ:
                        rhs = pb_tiles[(p, j)][:, q0 - c0:q1 - c0]
                    else:
                        g = j // GS
                        lo = SPLOC[(p, j)] + (q0 - c0)
                        rhs = sp_tiles[(p, g)][:, lo:lo + (q1 - q0)]
                    nc.tensor.matmul(
                        outp[lb][:, q0 - 512 * qb:q1 - 512 * qb],
                        lhsT=vp[:, j, :], rhs=rhs,
                        start=(j == 0),
                        stop=(j == min(4 * qb + 3, NKT - 1)))
                return run

            def mk_drain(lb):
                def run():
                    qb = 2 * p + lb
                    osb = osb_pool.tile([DP1, 512], f32, tag="osb",
                                        name=f"osb_h{h}p{p}b{lb}")
                    nc.vector.tensor_copy(osb[:], outp[lb][:])
                    nc.sync.dma_start(out_d[h, :, 512 * qb:512 * (qb + 1)],
                                      osb[:])
                return run

            for j in pass_tiles(p):
                for lb in (0, 1):
                    qb = 2 * p + lb
                    if j > min(4 * qb + 3, NKT - 1):
                        continue
                    units.append(mk_mm(j, lb))
                    if j == min(4 * qb + 3, NKT - 1):
                        units.append(mk_drain(lb))
            return units

        def interleave(qk, pv):
            """Emit QK units with prior-section pv callables spread between."""
            done = 0
            for i, u in enumerate(qk):
                u()
                want = (i + 1) * len(pv) // len(qk)
                while done < want:
                    pv[done]()
                    done += 1
            while done < len(pv):
                pv[done]()
                done += 1

        prev_pv = []
        for h in range(HPC):
            qt, kt, vp = head_tiles.pop(h, None) or load_head(h)
            sp_tiles, pb_tiles = {}, {}
            for p in (0, 1):
                interleave(qk_units(h, p, qt, kt, vp, sp_tiles, pb_tiles),
                           prev_pv)
                prev_pv = pv_units(h, p, vp, sp_tiles, pb_tiles)
            if h + 1 < HPC:
                head_tiles[h + 1] = load_head(h + 1)
        for u in prev_pv:
            u()

    nc.compile()
    return nc


def prep_inputs(query, key, value, attn_mask):
    """Host-side layout prep (transposes/retiling/casts only) -> 8 in_maps."""
    query = np.asarray(query, dtype=np.float32)
    key = np.asarray(key, dtype=np.float32)
    value = np.asarray(value, dtype=np.float32)
    attn_mask = np.asarray(attn_mask).astype(bool)

    # fold the 1/sqrt(D)=0.125 softmax scale into Q
    qT = np.ascontiguousarray(
        (query * 0.125).transpose(0, 1, 3, 2)).astype(np.float16)
    kT = np.ascontiguousarray(key.transpose(0, 1, 3, 2)).astype(np.float16)

    vp = np.concatenate(
        [value, np.ones((B, H, S, 1), np.float32)], axis=3).astype(np.float16)
    # [B, H, S, 65] -> [B, H, 128, NKT, 65] (partition-contiguous tiles)
    vp = np.ascontiguousarray(
        vp.reshape(B, H, NKT, P, DP1).transpose(0, 1, 3, 2, 4))

    tril = np.tril(np.ones((S, S), dtype=bool))
    in_maps = []
    for b in range(B):
        m = (attn_mask[b] & tril)          # [q, k] True = keep
        mT = m.T                           # [k, q]
        m01 = np.empty((P, M01W), np.float16)
        for (p, j), o in M01OFF.items():
            c0, e = pchunk(j, p)
            m01[:, o:o + e - c0] = mT[P * j:P * (j + 1), c0:e]
        mb = np.empty((P, MBW), np.float16)
        for (p, j), o in MBOFF.items():
            c0, e = pchunk(j, p)
            mb[:, o:o + e - c0] = np.where(mT[P * j:P * (j + 1), c0:e],
                                           np.float16(0.0), np.float16(-100.0))
        for cl in range(NCORES // B):
            h0 = cl * HPC
            in_maps.append({
                "qt": np.ascontiguousarray(qT[b, h0:h0 + HPC]),
                "kt": np.ascontiguousarray(kT[b, h0:h0 + HPC]),
                "vp": np.ascontiguousarray(vp[b, h0:h0 + HPC]),
                "m01": m01,
                "mb": mb,
            })
    return in_maps


def run(query, key, value, attn_mask, trace=False, trace_cores=None):
    from concourse import bass_utils

    if "nc" not in _cache:
        _cache["nc"] = build_nc()
    nc = _cache["nc"]

    in_maps = prep_inputs(query, key, value, attn_mask)
    res = bass_utils.run_bass_kernel_spmd(
        nc, in_maps, core_ids=list(range(NCORES)),
        trace=trace, trace_cores=trace_cores)

    out = np.empty((B, H, S, D), np.float32)
    for c in range(NCORES):
        b = c // (NCORES // B)
        h0 = (c % (NCORES // B)) * HPC
        outt = res.results[c]["outt"]          # [HPC, 65, S]
        num = outt[:, 0:D, :]                  # [HPC, 64, S]
        den = outt[:, D:D + 1, :]              # [HPC, 1, S]
        out[b, h0:h0 + HPC] = (num / den).transpose(0, 2, 1)
    return out, res


def kernel(query, key, value, attn_mask):
    out, _ = run(query, key, value, attn_mask)
    return out


# revision 25
# speedup vs baseline: 1.1257x; 1.0034x over previous
# Trainium2 Bass kernel for masked causal attention
#   B=2, H=16, S=2048, D=64, bool attn_mask [B, S, S] + causal, softmax, @V.
#
# Sharding: 8 cores x 4 heads (cores 0-3 -> batch 0, cores 4-7 -> batch 1).
#
# Softmax numerator/denominator are computed unnormalized on device (ones-row
# in the PV lhsT gives the denominator row); the final divide happens on HOST.
#
# Each head is processed in TWO q-passes of 1024 columns. That keeps the PV
# accumulator at 2 PSUM banks (instead of 4), freeing one bank as a target
# for dependency-free "HAM keep-alive" dummy matmuls: the PE's clock gate
# (HAM) re-throttles 2.4GHz -> 1.2GHz whenever the PE micro-idles, and the
# score-ring WAR stalls (QK chunk N waits for the drain of chunk N-2) were
# costing ~50us/core of cold-clock matmuls. A dummy matmul emitted after
# every QK unit keeps the array active across those stalls.
#
# Per (head, pass, k-tile j), chunk = [max(128j, 1024p), 1024(p+1)):
#  - k-tiles 0..3 ("path B"): ACT exp reads the f32 score chunk straight
#    from PSUM; the 0/1 mask is applied multiplicatively on fp16 (GPSIMD for
#    j<2, DVE for j 2..3).
#  - k-tiles 4..15 ("path A"): DVE drains PSUM with a fused additive mask
#    bias (0 keep / -100 masked) into causal-packed fp16 group buffers; ACT
#    exps each group in ONE wide in-place instruction.
# PV accumulates [V | ones] @ p per 512-wide q-bank into f32 PSUM; banks
# drain via DVE as they complete. PV matmuls of a section are interleaved
# into the next section's QK stream to keep the PE dense.

import os
import numpy as np

B, H, S, D = 2, 16, 2048, 64
NCORES = 8
HPC = 4          # heads per core
P = 128
NKT = S // P     # 16 k-tiles
DP1 = D + 1      # 64 value rows + denominator ones-row
GS = 4           # k-tiles per wide-exp group (path A)
BJ = 4           # k-tiles on path B
PASS_W = 1024
GPSIMD_BJ = 4    # path-B k-tiles j < this get their mask mult on GPSIMD

PADQ = int(os.environ.get("ATTN_PADQ", "512"))   # dummy-MM cols per QK unit
PADN = int(os.environ.get("ATTN_PADN", "2"))     # dummy MMs per QK unit


def pass_tiles(p):
    return [j for j in range(NKT) if P * j < PASS_W * (p + 1)]


def pchunk(j, p):
    return (max(P * j, PASS_W * p), PASS_W * (p + 1))


# causal-packed mask layouts, per (pass, k-tile)
M01OFF, MBOFF, SPW, SPLOC = {}, {}, {}, {}
_off = 0
for _p in (0, 1):
    for _j in range(BJ):
        c0, e = pchunk(_j, _p)
        M01OFF[(_p, _j)] = _off
        _off += e - c0
M01W = _off                                       # 7424
_off = 0
for _p in (0, 1):
    for _j in pass_tiles(_p):
        if _j < BJ:
            continue
        c0, e = pchunk(_j, _p)
        g = _j // GS
        if (_p, g) not in SPW:
            SPW[(_p, g)] = 0
        MBOFF[(_p, _j)] = _off
        SPLOC[(_p, _j)] = SPW[(_p, g)]
        SPW[(_p, g)] += e - c0
        _off += e - c0
MBW = _off                                        # 9984

_cache = {}


def build_nc():
    import concourse.bacc as bacc
    import concourse.mybir as mybir
    import concourse.tile as tile
    from concourse import library_config
    from contextlib import ExitStack

    fp16 = mybir.dt.float16
    f32 = mybir.dt.float32
    Exp = mybir.ActivationFunctionType.Exp

    nc = bacc.Bacc("TRN2", target_bir_lowering=False, debug=False,
                   num_devices=NCORES)

    qt_d = nc.dram_tensor("qt", [HPC, D, S], fp16, kind="ExternalInput")
    kt_d = nc.dram_tensor("kt", [HPC, D, S], fp16, kind="ExternalInput")
    vp_d = nc.dram_tensor("vp", [HPC, P, NKT, DP1], fp16, kind="ExternalInput")
    m01_d = nc.dram_tensor("m01", [P, M01W], fp16, kind="ExternalInput")
    mb_d = nc.dram_tensor("mb", [P, MBW], fp16, kind="ExternalInput")
    out_d = nc.dram_tensor("outt", [HPC, DP1, S], f32, kind="ExternalOutput")

    with tile.TileContext(nc) as tc, ExitStack() as ctx:
        mk_pool = ctx.enter_context(tc.tile_pool(name="mk", bufs=1))
        qk_pool = ctx.enter_context(tc.tile_pool(name="qk", bufs=2))
        vp_pool = ctx.enter_context(tc.tile_pool(name="vpool", bufs=2))
        sp_pool = ctx.enter_context(tc.tile_pool(name="sp", bufs=2))
        pb_pool = ctx.enter_context(tc.tile_pool(name="pb", bufs=8))
        osb_pool = ctx.enter_context(tc.tile_pool(name="osb", bufs=4))
        warm_pool = ctx.enter_context(tc.tile_pool(name="warm", bufs=1))
        st_psum = ctx.enter_context(tc.tile_pool(name="st", bufs=2, space="PSUM"))
        o_psum = ctx.enter_context(tc.tile_pool(name="outp", bufs=1, space="PSUM"))

        nc.gpsimd.load_library(library_config.standard)

        # dummy bank: PE-only, nothing reads it -> dummy MMs have no deps
        wsb = warm_pool.tile([P, 512], fp16, tag="warm")
        nc.vector.memset(wsb[:], 0.0)
        dummy_ps = o_psum.tile([P, 512], f32, tag="dummy")
        # PE warm-up: dense matmuls on zeros open the HAM gate at kernel start
        for _ in range(8):
            nc.tensor.matmul(dummy_ps[:], lhsT=wsb[:, 0:128], rhs=wsb[:],
                             start=True, stop=True)

        def pad(n=None):
            for _ in range(PADN if n is None else n):
                if PADQ:
                    nc.tensor.matmul(dummy_ps[:, 0:PADQ],
                                     lhsT=wsb[0:64, 0:128],
                                     rhs=wsb[0:64, 0:PADQ],
                                     start=True, stop=True)

        def load_head(h):
            qt = qk_pool.tile([D, S], fp16, tag="qt")
            nc.sync.dma_start(qt[:], qt_d[h])
            kt = qk_pool.tile([D, S], fp16, tag="kt")
            nc.sync.dma_start(kt[:], kt_d[h])
            vp = vp_pool.tile([P, NKT, DP1], fp16, tag="vp")
            nc.sync.dma_start(vp[:], vp_d[h])
            return qt, kt, vp

        head_tiles = {0: load_head(0)}
        m01_sb = mk_pool.tile([P, M01W], fp16, tag="m01")
        mb_sb = mk_pool.tile([P, MBW], fp16, tag="mb")
        for (p_, j_), o_ in sorted(M01OFF.items(), key=lambda kv: kv[1]):
            c0, e = pchunk(j_, p_)
            nc.sync.dma_start(m01_sb[:, o_:o_ + e - c0], m01_d[:, o_:o_ + e - c0])
        for (p_, j_), o_ in sorted(MBOFF.items(), key=lambda kv: kv[1]):
            c0, e = pchunk(j_, p_)
            nc.sync.dma_start(mb_sb[:, o_:o_ + e - c0], mb_d[:, o_:o_ + e - c0])

        def qk_units(h, p, qt, kt, vp, sp_tiles, pb_tiles):
            """One callable per k-tile chunk of this pass."""
            def unit(j):
                def run():
                    c0, e = pchunk(j, p)
                    w = e - c0
                    st = st_psum.tile([P, PASS_W], f32, tag="st",
                                      name=f"st_h{h}p{p}j{j}")
                    for lo in range(0, w, 512):
                        wl = min(512, w - lo)
                        nc.tensor.matmul(
                            st[:, lo:lo + wl],
                            lhsT=kt[:, j * P:(j + 1) * P],
                            rhs=qt[:, c0 + lo:c0 + lo + wl],
                            start=True, stop=True)
                    pad()
                    if j < BJ:
                        # path B: chunk exp from PSUM, then 0/1 mask mult
                        pb = pb_pool.tile([P, PASS_W], fp16, tag="pb",
                                          name=f"pb_h{h}p{p}j{j}")
                        nc.scalar.activation(pb[:, :w], st[:, :w], Exp)
                        mo = M01OFF[(p, j)]
                        eng = nc.gpsimd if j < GPSIMD_BJ else nc.vector
                        eng.tensor_mul(pb[:, :w], pb[:, :w],
                                       m01_sb[:, mo:mo + w])
                        pb_tiles[(p, j)] = pb
                    else:
                        # path A: fused drain + additive mask bias
                        g = j // GS
                        if (p, g) not in sp_tiles:
                            sp_tiles[(p, g)] = sp_pool.tile(
                                [P, SPW[(p, g)]], fp16, tag=f"sp{p}{g}",
                                name=f"sp_h{h}p{p}g{g}")
                        lo = SPLOC[(p, j)]
                        mo = MBOFF[(p, j)]
                        nc.vector.tensor_add(sp_tiles[(p, g)][:, lo:lo + w],
                                             st[:, :w],
                                             mb_sb[:, mo:mo + w])
                        if j == max(jj for jj in pass_tiles(p)
                                    if jj // GS == g):
                            # one wide in-place exp for the whole group
                            nc.scalar.activation(sp_tiles[(p, g)][:],
                                                 sp_tiles[(p, g)][:], Exp)
                return run
            return [unit(j) for j in pass_tiles(p)]

        def pv_units(h, p, vp, sp_tiles, pb_tiles):
            """PV MMs (j ascending => per-bank start..stop order) plus the
            per-bank drain right after the bank's last MM."""
            outp = {}
            units = []

            def mk_mm(j, lb):
                def run():
                    qb = 2 * p + lb
                    if lb not in outp:
                        outp[lb] = o_psum.tile([DP1, 512], f32,
                                               tag=f"outp{lb}",
                                               name=f"outp_h{h}p{p}b{lb}")
                    q0 = max(P * j, 512 * qb)
                    q1 = 512 * (qb + 1)
                    c0 = pchunk(j, p)[0]
                    if j < BJ:
                        rhs = pb_tiles[(p, j)][:, q0 - c0:q1 - c0]
                    else:
                        g = j // GS
                        lo = SPLOC[(p, j)] + (q0 - c0)
                        rhs = sp_tiles[(p, g)][:, lo:lo + (q1 - q0)]
                    nc.tensor.matmul(
                        outp[lb][:, q0 - 512 * qb:q1 - 512 * qb],
                        lhsT=vp[:, j, :], rhs=rhs,
                        start=(j == 0),
                        stop=(j == min(4 * qb + 3, NKT - 1)))
                return run

            def mk_drain(lb):
                def run():
                    qb = 2 * p + lb
                    osb = osb_pool.tile([DP1, 512], f32, tag="osb",
                                        name=f"osb_h{h}p{p}b{lb}")
                    nc.vector.tensor_copy(osb[:], outp[lb][:])
                    nc.sync.dma_start(out_d[h, :, 512 * qb:512 * (qb + 1)],
                                      osb[:])
                return run

            for j in pass_tiles(p):
                for lb in (0, 1):
                    qb = 2 * p + lb
                    if j > min(4 * qb + 3, NKT - 1):
                        continue
                    units.append(mk_mm(j, lb))
                    if j == min(4 * qb + 3, NKT - 1):
                        units.append(mk_drain(lb))
            return units

        def interleave(qk, pv):
            """Emit QK units with prior-section pv callables spread between."""
            done = 0
            for i, u in enumerate(qk):
                u()
                want = (i + 1) * len(pv) // len(qk)
                while done < want:
                    pv[done]()
                    done += 1
            while done < len(pv):
                pv[done]()
                done += 1

        prev_pv = []
        for h in range(HPC):
            qt, kt, vp = head_tiles.pop(h, None) or load_head(h)
            sp_tiles, pb_tiles = {}, {}
            for p in (0, 1):
                interleave(qk_units(h, p, qt, kt, vp, sp_tiles, pb_tiles),
                           prev_pv)
                prev_pv = pv_units(h, p, vp, sp_tiles, pb_tiles)
            if h + 1 < HPC:
                head_tiles[h + 1] = load_head(h + 1)
        for u in prev_pv:
            u()

    nc.compile()
    return nc


def prep_inputs(query, key, value, attn_mask):
    """Host-side layout prep (transposes/retiling/casts only) -> 8 in_maps."""
    query = np.asarray(query, dtype=np.float32)
    key = np.asarray(key, dtype=np.float32)
    value = np.asarray(value, dtype=np.float32)
    attn_mask = np.asarray(attn_mask).astype(bool)

    # fold the 1/sqrt(D)=0.125 softmax scale into Q
    qT = np.ascontiguousarray(
        (query * 0.125).transpose(0, 1, 3, 2)).astype(np.float16)
    kT = np.ascontiguousarray(key.transpose(0, 1, 3, 2)).astype(np.float16)

    vp = np.concatenate(
        [value, np.ones((B, H, S, 1), np.float32)], axis=3).astype(np.float16)
    # [B, H, S, 65] -> [B, H, 128, NKT, 65] (partition-contiguous tiles)
    vp = np.ascontiguousarray(
        vp.reshape(B, H, NKT, P, DP1).transpose(0, 1, 3, 2, 4))

    tril = np.tril(np.ones((S, S), dtype=bool))
    in_maps = []
    for b in range(B):
        m = (attn_mask[b] & tril)          # [q, k] True = keep
        mT = m.T                           # [k, q]
        m01 = np.empty((P, M01W), np.float16)
        for (p, j), o in M01OFF.items():
            c0, e = pchunk(j, p)
            m01[:, o:o + e - c0] = mT[P * j:P * (j + 1), c0:e]
        mb = np.empty((P, MBW), np.float16)
        for (p, j), o in MBOFF.items():
            c0, e = pchunk(j, p)
            mb[:, o:o + e - c0] = np.where(mT[P * j:P * (j + 1), c0:e],
                                           np.float16(0.0), np.float16(-100.0))
        for cl in range(NCORES // B):
            h0 = cl * HPC
            in_maps.append({
                "qt": np.ascontiguousarray(qT[b, h0:h0 + HPC]),
                "kt": np.ascontiguousarray(kT[b, h0:h0 + HPC]),
                "vp": np.ascontiguousarray(vp[b, h0:h0 + HPC]),
                "m01": m01,
                "mb": mb,
            })
    return in_maps


def run(query, key, value, attn_mask, trace=False, trace_cores=None):
    from concourse import bass_utils

    if "nc" not in _cache:
        _cache["nc"] = build_nc()
    nc = _cache["nc"]

    in_maps = prep_inputs(query, key, value, attn_mask)
    res = bass_utils.run_bass_kernel_spmd(
        nc, in_maps, core_ids=list(range(NCORES)),
        trace=trace, trace_cores=trace_cores)

    out = np.empty((B, H, S, D), np.float32)
    for c in range(NCORES):
        b = c // (NCORES // B)
        h0 = (c % (NCORES // B)) * HPC
        outt = res.results[c]["outt"]          # [HPC, 65, S]
        num = outt[:, 0:D, :]                  # [HPC, 64, S]
        den = outt[:, D:D + 1, :]              # [HPC, 1, S]
        out[b, h0:h0 + HPC] = (num / den).transpose(0, 2, 1)
    return out, res


def kernel(query, key, value, attn_mask):
    out, _ = run(query, key, value, attn_mask)
    return out
